# revision 26
# baseline (speedup 1.0000x reference)
import numpy as np
import ml_dtypes

import concourse.bacc as bacc
import concourse.tile as tile
from concourse import mybir
from concourse.bass_utils import run_bass_kernel_spmd

F32 = mybir.dt.float32
F32R = mybir.dt.float32r
BF16 = mybir.dt.bfloat16
F8 = mybir.dt.float8e4
AF = mybir.ActivationFunctionType
ALU = mybir.AluOpType

NB, H, C, HID, EMB = 4, 12, 1024, 768, 512
E, M, L, SPAN = 22, 4, 16, 32
TD, INTER = 20, 256
NN = E + E * M + L
NREL, NLAYERS = 3, 4
EM, EMH, HS, LS = E * M, E * M * H, H * SPAN, L * SPAN
EE = E * E
PADW = 26 * 26
XSPC = 776
NTY = 3 * (NREL + 1)
W8S = 16.0
N_CORES = 8


def _build_adj():
    A = np.zeros((NREL, NN, NN), np.float32)
    for e in range(E):
        for m in range(M):
            mi = E + e * M + m
            A[0, e, mi] = A[0, mi, e] = 1.0
            for m2 in range(M):
                if m2 != m:
                    A[1, mi, E + e * M + m2] = 1.0
            li = E + E * M + ((e * M + m) % L)
            A[2, mi, li] = A[2, li, mi] = 1.0
    A = A / (A.sum(-1, keepdims=True) + 1e-5)
    return A


_TYPES = np.concatenate([np.zeros(E, np.int32), np.ones(EM, np.int32),
                         np.full(L, 2, np.int32)])

_KC1 = [(0, 128), (128, 128), (256, 128), (384, 128)]


def _constb_layout():
    lay = {}
    c = 0

    def add(nm, cols):
        nonlocal c
        lay[nm] = (c, cols)
        c += cols
    for kc in range(6):
        add(f"wtr{kc}", EMB)
    add("brow", EMB)
    add("brow8", EMB)
    add("onesrow", 128)
    add("onescol", 1)
    add("g2T", E)
    for kc in range(4):
        add(f"sumT{kc}", L)
    cba = c
    add("ptT", NN)
    add("tcat", EMB)
    return lay, c, cba


def _const8_layout():
    lay = {}
    c = 0

    def add(nm, cols):
        nonlocal c
        lay[nm] = (c, cols)
        c += cols
    for kc in range(4):
        add(f"fsw1T{kc}", INTER)
    for kc in range(4):
        add(f"fcw1T{kc}", INTER)
    for kc in range(2):
        add(f"fsw2T{kc}", EMB)
    for kc in range(2):
        add(f"fcw2T{kc}", EMB)
    return lay, c


def _constf_layout():
    lay = {}
    c = 0

    def add(nm, cols):
        nonlocal c
        lay[nm] = (c, cols)
        c += cols
    for nm, nch in (("ses1", 2), ("seb1", 2), ("fcs1", 2), ("fcb1", 2),
                    ("ses2", 4), ("seb2", 4), ("fcs2", 4), ("fcb2", 4)):
        for kc in range(nch):
            add(f"{nm}{kc}", 1)
    add("b1h", 1)
    add("b2h", 1)
    add("b3h0", 1)
    add("b3h1", 1)
    add("mtop", 1)
    add("mbot", 1)
    add("identf", 128)
    return lay, c


def _actb_layout():
    lay = {}
    c = 0

    def add(nm, cols):
        nonlocal c
        lay[nm] = (c, cols)
        c += cols
    for kc in range(6):
        add(f"xmT{kc}", EM)
    for mc in range(4):
        add(f"xsp{mc}", XSPC)
    for kc in range(3):
        add(f"attl{kc}", LS)
    return lay, c


_LAY_B, _CB, _CBA = _constb_layout()
_LAY_8, _C8 = _const8_layout()
_LAY_F, _CF = _constf_layout()
_LAY_A, _CA = _actb_layout()


def build_program(solo=False, stages=4):
    nc = bacc.Bacc("TRN2", target_bir_lowering=False, debug=False)

    def din(name, shape, dt=BF16):
        return nc.dram_tensor(name, list(shape), dt, kind="ExternalInput").ap()

    constb_d = din("constb", [128, _CB])
    const8_d = din("const8", [128, _C8], F8)
    constf_d = din("constf", [128, _CF], F32)
    actb_d = din("actb", [128, _CA])
    xp_d = din("xp", [128, 8 * HID])
    amp_d = din("amp", [128, 9 * C], F8)
    gTb_d = din("gTb", [128, 9 * 32], F8)
    wstp_d = [din(f"wstp{i}", [128, 16 * EMB]) for i in range(4)]
    w1sb_d = din("w1sb", [4, 128, 25 * 128])
    w2sb_d = din("w2sb", [2, 128, 25 * 128])
    w3sb_d = din("w3sb", [2, 128, 25 * 256])
    aallTb_d = din("aallTb", [NN, (NREL + 1) * NN])
    identb_d = din("identb", [128, 128])

    out_d = nc.dram_tensor("out", [256, EE], F32, kind="ExternalOutput").ap()

    groups = [[0, 1], [2, 3], [4, 5], [6, 7]]

    with tile.TileContext(nc) as tc:
      with tc.tile_pool(name="pconst", bufs=1) as pconst, \
           tc.tile_pool(name="pwork", bufs=1) as pwork, \
           tc.tile_pool(name="pdram", bufs=1, space="DRAM") as pdram:

        constb = pconst.tile([128, _CB], BF16)
        const8 = pconst.tile([128, _C8], F8)
        constf = pconst.tile([128, _CF], F32)
        identb = pconst.tile([128, 128], BF16)
        aallTb = pconst.tile([NN, (NREL + 1) * NN], BF16)
        aallM = pconst.tile([EM, (NREL + 1) * NN], BF16)
        aallL = pconst.tile([L, (NREL + 1) * NN], BF16)
        wstp_t = [pconst.tile([128, 16 * EMB], BF16, tag=f"wstp{l}",
                              name=f"wstp{l}") for l in range(4)]
        w1 = [pconst.tile([128, 25 * 128], BF16, tag=f"w1_{kc}",
                          name=f"w1_{kc}") for kc in range(4)]

        def cb(nm, rows=128):
            c0, cols = _LAY_B[nm]
            return constb[0:rows, c0:c0 + cols]

        def c8(nm, rows=128):
            c0, cols = _LAY_8[nm]
            return const8[0:rows, c0:c0 + cols]

        def cf(nm, rows=128):
            c0, cols = _LAY_F[nm]
            return constf[0:rows, c0:c0 + cols]

        wtr = [cb(f"wtr{kc}") for kc in range(6)]
        brow = cb("brow", rows=1)
        brow8 = cb("brow8", rows=8)
        onesrow = cb("onesrow", rows=1)
        onescol = cb("onescol")
        g2T = cb("g2T", rows=EM)
        sumT = [cb(f"sumT{kc}") for kc in range(4)]
        ptT = cb("ptT", rows=NTY)
        tcat = cb("tcat", rows=NTY)
        sev = {nm: [cf(f"{nm}{kc}") for kc in range(n)]
               for nm, n in (("ses1", 2), ("seb1", 2), ("fcs1", 2), ("fcb1", 2),
                             ("ses2", 4), ("seb2", 4), ("fcs2", 4),
                             ("fcb2", 4))}
        b1h = cf("b1h")
        b2h = cf("b2h")
        b3h = [cf("b3h0"), cf("b3h1")]
        ident = cf("identf")

        h0e = pwork.tile([E, EMB], BF16)
        h0m = pwork.tile([EM, EMB], BF16)
        h0l = pwork.tile([L, EMB], BF16)
        hfin = pwork.tile([NN, EMB], BF16)
        ectxT_sb = [pwork.tile([128, E], F32, tag=f"ectxT{i}", name=f"ectxT{i}")
                    for i in range(4)]
        ecT = [pwork.tile([128, E], F32R, tag=f"ecT{i}", name=f"ecT{i}")
               for i in range(4)]
        warm = pwork.tile([128, 512], BF16)
        nc.vector.memset(warm[:], 0.0)
        fusedp = [pwork.tile([128, PADW], BF16, tag=f"fusedp{i}",
                             name=f"fusedp{i}") for i in range(4)]
        g1pc = pwork.tile([128, 2 * PADW], BF16, tag="g1pc", name="g1pc")
        g2pc = pwork.tile([128, 2 * PADW], BF16, tag="g2pc", name="g2pc")
        g1p = [g1pc[:, i * PADW:(i + 1) * PADW] for i in range(2)]
        g2p = [g2pc[:, i * PADW:(i + 1) * PADW] for i in range(2)]
        for t_ in fusedp:
            nc.vector.memset(t_[:], 0.0)
        nc.vector.memset(g1pc[:], 0.0)
        nc.vector.memset(g2pc[:], 0.0)

        with tc.tile_pool(name="pbig", bufs=1) as pbig:
            gTb = pbig.tile([128, 9 * 32], F8)
            amp = pbig.tile([128, 9 * C], F8)
            xp = pbig.tile([128, 8 * HID], BF16)
            actb = pbig.tile([128, _CA], BF16)

            nc.scalar.dma_start(constf[:], constf_d[:])
            nc.scalar.dma_start(identb[:], identb_d[:])
            xm_cols = 6 * EM
            nc.gpsimd.dma_start(actb[:, 0:xm_cols], actb_d[:, 0:xm_cols])
            nc.gpsimd.dma_start(constb[:, 0:_CBA], constb_d[:, 0:_CBA])
            sp_cols = xm_cols + 4 * XSPC
            nc.gpsimd.dma_start(actb[:, xm_cols:sp_cols],
                                actb_d[:, xm_cols:sp_cols])
            nc.gpsimd.dma_start(actb[:, sp_cols:_CA], actb_d[:, sp_cols:_CA])
            nc.scalar.dma_start(aallM[:], aallTb_d[E:E + EM, :])
            nc.scalar.dma_start(aallL[:], aallTb_d[E + EM:NN, :])
            nc.scalar.dma_start(aallTb[:], aallTb_d[:])
            BL = (NREL + 1) * EMB

            def wstp_dma(layer):
                for hh in range(2):
                    nc.gpsimd.dma_start(
                        wstp_t[layer][:, hh * 2 * BL:(hh + 1) * 2 * BL],
                        wstp_d[layer][:, hh * 2 * BL:(hh + 1) * 2 * BL])
            nc.gpsimd.dma_start(constb[:, _CBA:_CB], constb_d[:, _CBA:_CB])
            wstp_dma(0)
            wstp_dma(1)
            nc.scalar.dma_start(gTb[:], gTb_d[:])
            for g in range(2):
                nc.gpsimd.dma_start(amp[:, g * 4608:(g + 1) * 4608],
                                    amp_d[:, g * 4608:(g + 1) * 4608])
            nc.gpsimd.dma_start(xp[:], xp_d[:])
            wstp_dma(2)
            wstp_dma(3)
            nc.gpsimd.dma_start(const8[:], const8_d[:])
            for kc in range(0, 4, 2):
                nc.gpsimd.dma_start(w1[kc][:], w1sb_d[kc])
                nc.gpsimd.dma_start(w1[kc + 1][:], w1sb_d[kc + 1])

            expm = pbig.tile([EM, EMB], BF16)
            wsb = [pbig.tile([128, 1], F32, tag=f"wsb{i}", name=f"wsb{i}")
                   for i in range(4)]
            wsum = [pbig.tile([128, L], BF16, tag=f"wsum{i}", name=f"wsum{i}")
                    for i in range(4)]

            def ca(nm, rows=128):
                c0, cols = _LAY_A[nm]
                return actb[0:rows, c0:c0 + cols]

            xmT = [ca(f"xmT{kc}") for kc in range(6)]
            xsp = [ca(f"xsp{mc}") for mc in range(4)]
            attl = [ca(f"attl{kc}") for kc in range(3)]

            with tc.tile_pool(name="ps1a", bufs=1, space="PSUM") as ps1a:
                jp = ps1a.tile([128, 512], F32, tag="jp", name="jp")
                for _ in range(14):
                    nc.tensor.matmul(jp[:], warm[:, 0:128], warm[:],
                                     start=True, stop=True)
                mrep_p = ps1a.tile([EM, EMB], F32, tag="mrep", name="mrep")
                for kc in range(6):
                    nc.tensor.matmul(mrep_p[:], xmT[kc][:, 0:EM], wtr[kc][:],
                                     start=(kc == 0), stop=False)
                nc.tensor.matmul(mrep_p[:], onesrow[0:1, 0:EM], brow[:],
                                 start=False, stop=True)
                nc.scalar.copy(h0m[:], mrep_p[:])
                nc.scalar.activation(expm[:], mrep_p[:], AF.Exp)
                ep_p = ps1a.tile([E, EMB], F32, tag="ep", name="ep")
                nc.tensor.matmul(ep_p[:], g2T[:], expm[:], start=True, stop=True)
                nc.scalar.activation(h0e[:], ep_p[:], AF.Ln)
                sigwarm = pbig.tile([1, 1], F32)
                nc.scalar.activation(sigwarm[:], ep_p[0:1, 0:1], AF.Sigmoid)

                for mc in range(4):
                    w_p = ps1a.tile([128, 1], F32, tag="w_p", name="w_p", bufs=1)
                    for kc in range(3):
                        nc.tensor.matmul(w_p[:],
                                         attl[kc][:, mc * 128:(mc + 1) * 128],
                                         onescol[:],
                                         start=(kc == 0), stop=(kc == 2))
                    nc.scalar.activation(wsb[mc][:], w_p[:], AF.Copy,
                                         scale=1.0 / (H * SPAN))
                    nc.vector.tensor_scalar(out=wsum[mc][:], in0=sumT[mc][:],
                                            scalar1=wsb[mc][:], scalar2=None,
                                            op0=ALU.mult)
                t_ps = [ps1a.tile([L, 388], F32, tag=f"t_p{i}", name=f"t_p{i}")
                        for i in range(2)]
                for mc in range(4):
                    for hh in range(2):
                        nc.tensor.matmul(t_ps[hh][:], wsum[mc][:],
                                         xsp[mc][:, hh * 388:(hh + 1) * 388],
                                         start=(mc == 0), stop=(mc == 3))
                t_sb = pbig.tile([L, XSPC], BF16)
                nc.scalar.copy(t_sb[:, 0:388], t_ps[0][:])
                nc.vector.tensor_copy(out=t_sb[:, 388:XSPC], in_=t_ps[1][:])
                link_p = ps1a.tile([L, EMB], F32, tag="link", name="link")
                for kc in range(6):
                    ttp = ps1a.tile([128, L], BF16, tag="ttp", name="ttp",
                                    bufs=1)
                    nc.tensor.transpose(ttp[:],
                                        t_sb[:, kc * 128:(kc + 1) * 128],
                                        identb[0:L, 0:L])
                    tts = pbig.tile([128, L], BF16, tag="tts", name="tts",
                                    bufs=2)
                    if kc % 2 == 0:
                        nc.scalar.copy(tts[:], ttp[:])
                    else:
                        nc.vector.tensor_copy(out=tts[:], in_=ttp[:])
                    nc.tensor.matmul(link_p[:], tts[:], wtr[kc][:],
                                     start=(kc == 0), stop=False)
                ttp8 = ps1a.tile([8, L], BF16, tag="ttp", name="ttp8", bufs=1)
                nc.tensor.transpose(ttp8[:], t_sb[:, 768:776],
                                    identb[0:L, 0:L])
                tts8 = pbig.tile([8, L], BF16)
                nc.scalar.copy(tts8[:], ttp8[:])
                nc.tensor.matmul(link_p[:], tts8[:], brow8[:],
                                 start=False, stop=True)
                nc.scalar.copy(h0l[:], link_p[:])

            ea_sb = pbig.tile([E, C], F32R)
            eaT = [pbig.tile([128, E], BF16, tag=f"eaT{i}", name=f"eaT{i}")
                   for i in range(8)]
            z_sb = [pbig.tile([128, E], BF16, tag=f"z{i}", name=f"z{i}")
                    for i in range(6)]
            easumT = pbig.tile([1, E], BF16)

            if stages >= 2:
              with tc.tile_pool(name="prg", bufs=2) as prg, \
                   tc.tile_pool(name="psr", bufs=1, space="PSUM") as psr:

                def rgcn_layer(layer, h):
                    kcs = _KC1
                    nk = len(kcs)
                    wstp = wstp_t[layer]
                    wst_t = [wstp[:, (si * (NREL + 1) + r) * EMB:
                                   (si * (NREL + 1) + r + 1) * EMB]
                             for r in range(NREL + 1) for si in range(nk)]
                    u_sb = []
                    for si, (s0, sl) in enumerate(kcs):
                        u_p = psr.tile([128, (NREL + 1) * NN], F32, tag="u_p",
                                       name="u_p", bufs=2)
                        if layer == 0:
                            nc.tensor.matmul(u_p[0:sl, :],
                                             h0e[:, s0:s0 + sl],
                                             aallTb[0:E, :],
                                             start=True, stop=False)
                            nc.tensor.matmul(u_p[0:sl, :],
                                             h0m[:, s0:s0 + sl], aallM[:],
                                             start=False, stop=False)
                            nc.tensor.matmul(u_p[0:sl, :],
                                             h0l[:, s0:s0 + sl], aallL[:],
                                             start=False, stop=True)
                        else:
                            nc.tensor.matmul(u_p[0:sl, :], h[0:NN, s0:s0 + sl],
                                             aallTb[:], start=True, stop=True)
                        u = prg.tile([128, (NREL + 1) * NN], BF16, tag=f"u{si}",
                                     name=f"u{si}", bufs=1)
                        if si % 2 == 0:
                            nc.scalar.copy(u[0:sl, :], u_p[0:sl, :])
                        else:
                            nc.vector.tensor_copy(out=u[0:sl, :],
                                                  in_=u_p[0:sl, :])
                        u_sb.append(u)
                    y_p = psr.tile([NN, EMB], F32, tag="y_p", name="y_p")
                    n_mm = (NREL + 1) * nk
                    k_mm = 0
                    last_closes = layer != 0
                    for si, (s0, sl) in enumerate(kcs):
                        for r in range(NREL + 1):
                            nc.tensor.matmul(
                                y_p[:], u_sb[si][0:sl, r * NN:(r + 1) * NN],
                                wst_t[r * nk + si][0:sl, :],
                                start=(k_mm == 0),
                                stop=(last_closes and k_mm == n_mm - 1))
                            k_mm += 1
                    if layer == 0:
                        nc.tensor.matmul(y_p[:], ptT[:], tcat[:],
                                         start=False, stop=True)
                    hdst = hfin if layer == NLAYERS - 1 else \
                        prg.tile([NN, EMB], BF16, tag="h_next", name="h_next")
                    for (s0, sl) in _KC1:
                        nc.scalar.activation(hdst[0:NN, s0:s0 + sl],
                                             y_p[0:NN, s0:s0 + sl], AF.Relu)
                    return hdst

                h1 = rgcn_layer(0, None)

                with tc.tile_pool(name="ps1b", bufs=1, space="PSUM") as ps1b:
                    ea_p0 = ps1b.tile([E, 512], F32, tag="ea0", name="ea0")
                    ea_p1 = ps1b.tile([E, 512], F32, tag="ea1", name="ea1")
                    ampv = amp[:].rearrange("p (k c) -> p k c", k=9)
                    gTv = gTb[:].rearrange("p (k e) -> p k e", k=9)
                    for kp in range(4):
                        gt = gTv[:, 2 * kp:2 * kp + 2, 0:E]
                        at = ampv[:, 2 * kp:2 * kp + 2, :]
                        nc.tensor.matmul(ea_p0[:], gt, at[:, :, 0:512],
                                         perf_mode=mybir.MatmulPerfMode.DoubleRow,
                                         start=(kp == 0), stop=False)
                        nc.tensor.matmul(ea_p1[:], gt, at[:, :, 512:1024],
                                         perf_mode=mybir.MatmulPerfMode.DoubleRow,
                                         start=(kp == 0), stop=False)
                    at8 = amp[0:32, 8 * C:9 * C]
                    gt8 = gTb[0:32, 8 * 32:8 * 32 + E]
                    nc.tensor.matmul(ea_p0[:], gt8, at8[:, 0:512],
                                     start=False, stop=True)
                    nc.tensor.matmul(ea_p1[:], gt8, at8[:, 512:1024],
                                     start=False, stop=True)
                    r0 = pbig.tile([E, 1], F32)
                    r1 = pbig.tile([E, 1], F32)
                    nc.vector.tensor_reduce(r0[:], ea_p0[:],
                                            mybir.AxisListType.X, ALU.add)
                    nc.vector.tensor_reduce(r1[:], ea_p1[:],
                                            mybir.AxisListType.X, ALU.add)
                    rsum = pbig.tile([E, 1], F32)
                    nc.vector.tensor_tensor(out=rsum[:], in0=r0[:], in1=r1[:],
                                            op=ALU.add)
                    rsum2 = pbig.tile([E, 1], F32)
                    nc.vector.tensor_scalar(out=rsum2[:], in0=rsum[:],
                                            scalar1=1e-5, scalar2=None,
                                            op0=ALU.add)
                    rinv = pbig.tile([E, 1], F32)
                    nc.vector.reciprocal(rinv[:], rsum2[:])
                    for kc in range(4):
                        c0, c1_ = kc * 128, (kc + 1) * 128
                        if kc % 2 == 0:
                            nc.scalar.copy(ea_sb[:, c0:c1_], ea_p0[:, c0:c1_])
                            nc.scalar.copy(ea_sb[:, 512 + c0:512 + c1_],
                                           ea_p1[:, c0:c1_])
                        else:
                            nc.vector.tensor_copy(out=ea_sb[:, c0:c1_],
                                                  in_=ea_p0[:, c0:c1_])
                            nc.vector.tensor_copy(
                                out=ea_sb[:, 512 + c0:512 + c1_],
                                in_=ea_p1[:, c0:c1_])
                    easum = pbig.tile([E, 1], F32)
                    nc.vector.tensor_tensor(out=easum[:], in0=rsum[:],
                                            in1=rinv[:], op=ALU.mult)
                    for kc in range(8):
                        tp = ps1b.tile([128, E], F32, tag=f"ea{kc % 2}",
                                       name="eaTt")
                        nc.tensor.transpose(tp[:],
                                            ea_sb[:, kc * 128:(kc + 1) * 128]
                                            .bitcast(F32), ident[0:E, 0:E])
                        if kc % 2 == 0:
                            nc.scalar.copy(eaT[kc][:], tp[:])
                        else:
                            nc.vector.tensor_copy(out=eaT[kc][:], in_=tp[:])
                    tp = ps1b.tile([1, E], F32, tag="ea1", name="easumt")
                    nc.tensor.transpose(tp[:], easum[:], ident[0:E, 0:E])
                    nc.scalar.copy(easumT[:], tp[:])

                h2 = rgcn_layer(1, h1)

                with tc.tile_pool(name="ps1c", bufs=1, space="PSUM") as ps1c:
                    zt_ps = [ps1c.tile([E, 384], F32, tag="sc",
                                       name=f"zt_p{i}", bufs=2)
                             for i in range(2)]
                    for kc in range(8):
                        xt = xp[:, kc * HID:(kc + 1) * HID]
                        for hh in range(2):
                            nc.tensor.matmul(zt_ps[hh][:], eaT[kc][:],
                                             xt[:, hh * 384:(hh + 1) * 384],
                                             start=(kc == 0), stop=(kc == 7))
                    zt_sb = pbig.tile([E, HID], F32)
                    nc.scalar.activation(zt_sb[:, 0:384], zt_ps[0][:], AF.Copy,
                                         scale=rinv[:])
                    nc.scalar.activation(zt_sb[:, 384:768], zt_ps[1][:],
                                         AF.Copy, scale=rinv[:])
                    for kc in range(6):
                        ztp = ps1c.tile([128, E], F32, tag="tp", name="ztp",
                                        bufs=1)
                        nc.tensor.transpose(ztp[:],
                                            zt_sb[:, kc * 128:(kc + 1) * 128],
                                            ident[0:E, 0:E])
                        if kc % 2 == 0:
                            nc.scalar.copy(z_sb[kc][:], ztp[:])
                        else:
                            nc.vector.tensor_copy(out=z_sb[kc][:], in_=ztp[:])
                    ec2_p = ps1c.tile([E, EMB], F32, tag="sc", name="ec2",
                                      bufs=2)
                    for kc in range(6):
                        nc.tensor.matmul(ec2_p[:], z_sb[kc][:], wtr[kc][:],
                                         start=(kc == 0), stop=False)
                    nc.tensor.matmul(ec2_p[:], easumT[:], brow[:],
                                     start=False, stop=True)
                    ec2_sb = pbig.tile([E, EMB], F32)
                    nc.scalar.copy(ec2_sb[:], ec2_p[:])
                    for mc in range(4):
                        ecp = ps1c.tile([128, E], F32, tag="tp", name="ecp",
                                        bufs=1)
                        nc.tensor.transpose(ecp[:],
                                            ec2_sb[:, mc * 128:(mc + 1) * 128],
                                            ident[0:E, 0:E])
                        if mc % 2 == 0:
                            nc.scalar.copy(ectxT_sb[mc][:], ecp[:])
                        else:
                            nc.vector.tensor_copy(out=ectxT_sb[mc][:],
                                                  in_=ecp[:])

                    h3 = rgcn_layer(2, h2)
                    rgcn_layer(3, h3)

                    for mc in range(4):
                        tp = ps1c.tile([128, E], F32,
                                       tag="tp" if mc % 2 == 0 else "sc",
                                       name="est", bufs=1 if mc % 2 == 0 else 2)
                        nc.tensor.matmul(tp[:],
                                         hfin[0:E, mc * 128:(mc + 1) * 128],
                                         identb[0:E, 0:E], start=True,
                                         stop=True)
                        nc.vector.tensor_tensor(out=ecT[mc][:], in0=tp[:],
                                                in1=ectxT_sb[mc][:],
                                                op=ALU.add)

        if stages >= 3:
          DR = mybir.MatmulPerfMode.DoubleRow
          fmap = [pwork.tile([128, EE], BF16, tag=f"fmap{i}", name=f"fmap{i}")
                  for i in range(4)]
          fmap8 = [pwork.tile([128, 2 * EE], F8, tag=f"fmap8_{i}",
                              name=f"fmap8_{i}") for i in range(2)]
          pooled8 = pwork.tile([128, 4], F8, tag="pool8", name="pool8")
          for mc in range(4):
              for ee, lo, hi in ((nc.vector, 0, 11), (nc.gpsimd, 11, E)):
                  o6v = fmap[mc][:].rearrange("p (i j) -> p i j", i=E)[:, lo:hi]
                  in0 = ecT[mc][:, lo:hi].rearrange("p (i j) -> p i j", j=1) \
                      .to_broadcast([128, hi - lo, E])
                  in1 = ecT[mc][:].rearrange("p (o j) -> p o j", o=1) \
                      .to_broadcast([128, hi - lo, E])
                  ee.tensor_tensor(out=o6v, in0=in0, in1=in1, op=ALU.mult)
              nc.scalar.copy(fmap8[mc // 2][:, (mc % 2) * EE:(mc % 2 + 1) * EE],
                             fmap[mc][:])
              rs = pwork.tile([128, 1], F32, tag=f"rs{mc}", name=f"rs{mc}")
              nc.vector.tensor_reduce(rs[:], ecT[mc][:], mybir.AxisListType.X,
                                      ALU.add)
              nc.scalar.activation(pooled8[:, mc:mc + 1], rs[:], AF.Square,
                                   scale=1.0 / E)

          pse_cm = tc.tile_pool(name="pse", bufs=1, space="PSUM")
          pse = pse_cm.__enter__()
          if True:
              def pair8(nm, block, col_off, width):
                  c0, _ = _LAY_8[nm]
                  return const8[:, c0:c0 + 2 * block].rearrange(
                      "p (k i) -> p k i", k=2)[:, :, col_off:col_off + width]

              c18 = pwork.tile([128, 2], F8, tag="c18", name="c18")
              for oc in range(2):
                  c1_p = pse.tile([128, 1], F32, tag="c1p", name="c1p")
                  for pp in range(2):
                      nc.tensor.matmul(
                          c1_p[:],
                          pair8(f"fcw1T{2*pp}", INTER, oc * 128, 128),
                          pooled8[:, 2 * pp:2 * pp + 2].rearrange(
                              "p (k n) -> p k n", k=2),
                          perf_mode=DR, start=(pp == 0), stop=(pp == 1))
                  nc.scalar.activation(c18[:, oc:oc + 1], c1_p[:], AF.Relu,
                                       bias=sev["fcb1"][oc][:],
                                       scale=sev["fcs1"][oc][:])
              cbb = [pwork.tile([128, 1], F32, tag=f"cbb{i}", name=f"cbb{i}")
                     for i in range(4)]
              c18v = c18[:].rearrange("p (k n) -> p k n", k=2)
              for mc in range(4):
                  c2_p = pse.tile([128, 1], F32, tag="c2p", name="c2p")
                  nc.tensor.matmul(
                      c2_p[:], pair8("fcw2T0", EMB, mc * 128, 128),
                      c18v, perf_mode=DR, start=True, stop=True)
                  nc.scalar.activation(cbb[mc][:], c2_p[:], AF.Identity,
                                       bias=sev["fcb2"][mc][:],
                                       scale=sev["fcs2"][mc][:])
              s18 = pwork.tile([128, 2 * EE], F8, tag="s18", name="s18")
              for oc in range(2):
                  s1_p = pse.tile([128, EE], F32, tag="s1p", name="s1p", bufs=2)
                  for pp in range(2):
                      nc.tensor.matmul(
                          s1_p[:],
                          pair8(f"fsw1T{2*pp}", INTER, oc * 128, 128),
                          fmap8[pp][:].rearrange("p (k n) -> p k n", k=2),
                          perf_mode=DR, start=(pp == 0), stop=(pp == 1))
                  nc.scalar.activation(s18[:, oc * EE:(oc + 1) * EE], s1_p[:],
                                       AF.Relu, bias=sev["seb1"][oc][:],
                                       scale=sev["ses1"][oc][:])
              for mc in range(4):
                  s2_p = pse.tile([128, EE], F32, tag="s2p", name="s2p", bufs=2)
                  nc.tensor.matmul(
                      s2_p[:], pair8("fsw2T0", EMB, mc * 128, 128),
                      s18[:].rearrange("p (k n) -> p k n", k=2),
                      perf_mode=DR, start=True, stop=True)
                  sig = pwork.tile([128, EE], BF16, tag="sig", name="sig",
                                   bufs=2)
                  nc.scalar.activation(sig[:], s2_p[:], AF.Sigmoid,
                                       bias=cbb[mc][:], scale=sev["ses2"][mc][:])
                  for ee, lo, hi in ((nc.vector, 0, 11), (nc.gpsimd, 11, E)):
                      outv = fusedp[mc][:].rearrange(
                          "p (i j) -> p i j", j=26)[:, 2 + lo:2 + hi, 2:24]
                      ee.tensor_tensor(
                          out=outv,
                          in0=fmap[mc][:].rearrange("p (i j) -> p i j",
                                                    i=E)[:, lo:hi],
                          in1=sig[:].rearrange("p (i j) -> p i j",
                                               i=E)[:, lo:hi],
                          op=ALU.mult)

        if stages >= 4:
          SLICES = [(0, 8), (8, 13), (13, 17), (17, 22)]
          RH = 11 * 22

          def tap_rows(padt, tap, r0, nr):
              dy, dx = tap // 5, tap % 5
              return padt.rearrange("p (i j) -> p i j", j=26)[
                  :, dy + r0:dy + r0 + nr, dx:dx + 22]

          def rd_pair(gpc, r0, nr):
              return gpc[:].rearrange("p (c i j) -> p c i j", c=2, j=26)[
                  :, :, 2 + r0:2 + r0 + nr, 2:24]

          with tc.tile_pool(name="pcw", bufs=1) as pcw:
              psc = pse
              w2 = []
              for kc in range(2):
                  t = pcw.tile([128, 25 * 128], BF16, tag=f"w2_{kc}",
                               name=f"w2_{kc}")
                  for ch in range(2):
                      nc.gpsimd.dma_start(t[:, ch * 1600:(ch + 1) * 1600],
                                          w2sb_d[kc][:, ch * 1600:(ch + 1) * 1600])
                  w2.append(t)
              w3 = []
              for kc in range(2):
                  t = pcw.tile([128, 25 * 256], BF16, tag=f"w3_{kc}",
                               name=f"w3_{kc}")
                  for ch in range(4):
                      nc.gpsimd.dma_start(t[:, ch * 1600:(ch + 1) * 1600],
                                          w3sb_d[kc][:, ch * 1600:(ch + 1) * 1600])
                  w3.append(t)

              def exchange_slice(stage_sb, dram_pre, gpc, slices, sl_i):
                  r0, r1_ = slices[sl_i]
                  nr = r1_ - r0
                  seg = stage_sb[:, r0 * 22:r1_ * 22]
                  gseg = pdram.tile([256, nr * 22], BF16,
                                    tag=f"{dram_pre}g{sl_i}",
                                    name=f"{dram_pre}g{sl_i}")
                  if solo:
                      nc.sync.dma_start(gseg[0:128, :], seg)
                      nc.sync.dma_start(gseg[128:256, :], seg)
                  else:
                      bseg = pdram.tile([128, nr * 22], BF16,
                                        tag=f"{dram_pre}b{sl_i}",
                                        name=f"{dram_pre}b{sl_i}")
                      nc.sync.dma_start(bseg[:], seg)
                      nc.gpsimd.collective_compute(
                          "AllGather", ALU.bypass, replica_groups=groups,
                          ins=[bseg[:].opt()], outs=[gseg[:].opt()])
                  gv = gpc[:].rearrange("p (c i j) -> p c i j", c=2, j=26)
                  nc.scalar.dma_start(gv[:, 0, 2 + r0:2 + r0 + nr, 2:24],
                                      gseg[0:128, :])
                  nc.gpsimd.dma_start(gv[:, 1, 2 + r0:2 + r0 + nr, 2:24],
                                      gseg[128:256, :])

              def conv_sliced(wsel, srcs, nkc, stage_sb, bias, dram_pre,
                              gpc, slices):
                  for sl_i, (r0, r1_) in enumerate(slices):
                      nr = r1_ - r0
                      cp = psc.tile([128, RH], F32, tag="cp", name="cp",
                                    bufs=2)
                      cpv = cp[:, 0:nr * 22]
                      k = 0
                      for kc in range(nkc):
                          for tap in range(25):
                              nc.tensor.matmul(
                                  cpv, wsel(kc, tap),
                                  tap_rows(srcs[kc], tap, r0, nr),
                                  start=(k == 0), stop=(k == 25 * nkc - 1))
                              k += 1
                      nc.scalar.activation(stage_sb[:, r0 * 22:r1_ * 22], cpv,
                                           AF.Relu, bias=bias)
                      exchange_slice(stage_sb, dram_pre, gpc, slices, sl_i)

              r1s = pcw.tile([128, EE], BF16, tag="r1s", name="r1s")
              conv_sliced(
                  lambda kc, tap: w1[kc][:, tap * 128:(tap + 1) * 128],
                  [t[:] for t in fusedp], 4, r1s, b1h[:], "r1", g1pc,
                  [(0, 8), (8, 13), (13, 17), (17, 22)])

              r2s = pcw.tile([128, EE], BF16, tag="r2s", name="r2s")
              conv_sliced(
                  lambda kc, tap: w2[kc][:, tap * 128:(tap + 1) * 128],
                  g1p, 2, r2s, b2h[:], "r2", g2pc,
                  [(0, 10), (10, 13), (13, 22)])

              for (oc, hh) in ((0, 0), (1, 0), (0, 1), (1, 1)):
                  last = (oc == 1 and hh == 1)
                  rows = [(0, 6), (6, 9), (9, 11)] if last else [(0, 11)]
                  for ri, (ra, rb) in enumerate(rows):
                      nr = rb - ra
                      cp = psc.tile([128, RH], F32, tag="cp", name="cp",
                                    bufs=2)
                      cpv = cp[:, 0:nr * 22]
                      order = ([t for t in range(25) if t // 5 <= 1] +
                               [t for t in range(25) if t // 5 > 1]) \
                          if hh == 0 else list(range(25))
                      k = 0
                      for tap in order:
                          for kc in range(2):
                              nc.tensor.matmul(
                                  cpv,
                                  w3[kc][:, tap * 256 + oc * 128:
                                         tap * 256 + (oc + 1) * 128],
                                  tap_rows(g2p[kc], tap, hh * 11 + ra, nr),
                                  start=(k == 0), stop=(k == 49))
                              k += 1
                      o_sb = pcw.tile([128, RH], F32, tag="osb",
                                      name="osb", bufs=3)
                      ov = o_sb[:, 0:nr * 22]
                      nc.scalar.activation(ov, cpv, AF.Relu, bias=b3h[oc][:])
                      eng = nc.sync if (oc + hh + ri) % 2 == 0 else nc.scalar
                      eng.dma_start(
                          out_d[oc * 128:(oc + 1) * 128,
                                hh * RH + ra * 22:hh * RH + rb * 22], ov)

        if stages >= 3:
            pse_cm.__exit__(None, None, None)

    nc.compile()
    return nc


_NC_CACHE = None


def _get_program():
    global _NC_CACHE
    if _NC_CACHE is None:
        _NC_CACHE = build_program()
    return _NC_CACHE


def _bf(a):
    return np.ascontiguousarray(a.astype(ml_dtypes.bfloat16))


def _prep_shared(w):
    ADJ = _build_adj()
    out = {}
    constb = np.zeros((128, _CB), np.float32)

    def put(nm, arr):
        c0, cols = _LAY_B[nm]
        r, cc = arr.shape
        constb[0:r, c0:c0 + cc] = arr
    wt = w['W_trans']
    for kc in range(6):
        put(f"wtr{kc}", wt[kc * 128:(kc + 1) * 128])
    put("brow", w['b_trans'].reshape(1, EMB))
    brow8 = np.zeros((8, EMB), np.float32)
    brow8[0] = w['b_trans']
    put("brow8", brow8)
    put("onesrow", np.ones((1, 128), np.float32))
    put("onescol", np.ones((128, 1), np.float32))
    g2T = np.zeros((EM, E), np.float32)
    for e in range(E):
        g2T[e * M:(e + 1) * M, e] = 1.0
    put("g2T", g2T)
    sumT = np.kron(np.eye(L, dtype=np.float32), np.ones((SPAN, 1), np.float32))
    for kc in range(4):
        put(f"sumT{kc}", sumT[kc * 128:(kc + 1) * 128])
    Q = np.zeros((NN, 3), np.float32)
    Q[np.arange(NN), _TYPES] = 1.0
    P = np.concatenate([ADJ[r] @ Q for r in range(NREL)] + [Q], axis=1)
    Tcat = np.concatenate(
        [w['type_embed'] @ w['rgcn_Wrel0'][r][EMB:EMB + TD]
         for r in range(NREL)] +
        [w['type_embed'] @ w['rgcn_Wself0'][EMB:EMB + TD]], axis=0)
    put("ptT", np.ascontiguousarray(P.T))
    put("tcat", Tcat)
    out['constb'] = _bf(constb)

    const8 = np.zeros((128, _C8), np.float32)

    def put8(nm, arr):
        c0, cols = _LAY_8[nm]
        const8[0:arr.shape[0], c0:c0 + arr.shape[1]] = arr
    for nm, arr, nch in (("fsw1T", w['fs_w1'].T, 4), ("fcw1T", w['fc_w1'].T, 4),
                         ("fsw2T", w['fs_w2'].T, 2), ("fcw2T", w['fc_w2'].T, 2)):
        for kc in range(nch):
            put8(f"{nm}{kc}",
                 np.ascontiguousarray(arr[kc * 128:(kc + 1) * 128]) * W8S)
    out['const8'] = np.ascontiguousarray(
        const8.astype(ml_dtypes.float8_e4m3))

    gT = np.zeros((EMH, E), np.float32)
    for e in range(E):
        gT[e * M * H:(e + 1) * M * H, e] = 1.0 / (M * H)
    gTb = np.zeros((128, 9 * 32), np.float32)
    for kc in range(9):
        r = min(128, EMH - kc * 128)
        gTb[0:r, kc * 32:kc * 32 + E] = gT[kc * 128:kc * 128 + r]
    out['gTb'] = np.ascontiguousarray(gTb.astype(ml_dtypes.float8_e4m3))
    out['aallTb'] = _bf(np.concatenate(
        [ADJ[r].T for r in range(NREL)] + [np.eye(NN, dtype=np.float32)],
        axis=1))
    out['identb'] = _bf(np.eye(128, dtype=np.float32))

    constf = np.zeros((128, _CF), np.float32)

    def putf(nm, arr):
        c0, cols = _LAY_F[nm]
        constf[0:arr.shape[0], c0:c0 + 1] = arr.reshape(-1, 1)
    s8 = 1.0 / W8S
    vecs = {"ses1": w['fs_g1'] * s8,
            "seb1": w['fs_b1'] * w['fs_g1'] + w['fs_be1'],
            "fcs1": w['fc_g1'] * s8,
            "fcb1": w['fc_b1'] * w['fc_g1'] + w['fc_be1'],
            "ses2": w['fs_g2'] * s8,
            "seb2": w['fs_b2'] * w['fs_g2'] + w['fs_be2'],
            "fcs2": w['fc_g2'] * s8,
            "fcb2": w['fc_b2'] * w['fc_g2'] + w['fc_be2'] +
                    w['fs_b2'] * w['fs_g2'] + w['fs_be2']}
    for nm, v in vecs.items():
        nch = 2 if v.shape[0] == INTER else 4
        for kc in range(nch):
            putf(f"{nm}{kc}", v[kc * 128:(kc + 1) * 128])
    out['constf_base'] = constf

    for layer in range(NLAYERS):
        din_l = EMB + TD if layer == 0 else EMB
        nk = len(_KC1)
        Wst = w['rgcn_Wrel0'].reshape(NREL * din_l, EMB) if layer == 0 else \
            w['rgcn_Wrel'][layer - 1].reshape(NREL * EMB, EMB)
        Wself = w['rgcn_Wself0'] if layer == 0 else w['rgcn_Wself'][layer - 1]
        p = np.zeros((128, (NREL + 1) * nk * EMB), np.float32)
        for si, (s0, sl) in enumerate(_KC1):
            for r in range(NREL):
                b = si * (NREL + 1) + r
                p[0:sl, b * EMB:(b + 1) * EMB] = \
                    Wst[r * din_l + s0:r * din_l + s0 + sl]
            b = si * (NREL + 1) + NREL
            p[0:sl, b * EMB:(b + 1) * EMB] = Wself[s0:s0 + sl]
        out[f'wstp{layer}'] = _bf(p)
    return out


def _prep_conv_half(w, half, constf_base):
    out = {}
    w1 = w['cr_w1'][half * 128:(half + 1) * 128]
    out['w1sb'] = _bf(np.ascontiguousarray(
        w1.transpose(1, 2, 3, 0).reshape(4, 128, 25 * 128)))
    w2 = w['cr_w2'][half * 128:(half + 1) * 128]
    out['w2sb'] = _bf(np.ascontiguousarray(
        w2.transpose(1, 2, 3, 0).reshape(2, 128, 25 * 128)))
    w3 = w['cr_w3'][half * 256:(half + 1) * 256]
    out['w3sb'] = _bf(np.ascontiguousarray(
        w3.transpose(1, 2, 3, 0).reshape(2, 128, 25 * 256)))
    constf = constf_base.copy()

    def putf(nm, arr):
        c0, cols = _LAY_F[nm]
        constf[0:arr.shape[0], c0:c0 + 1] = arr.reshape(-1, 1)
    putf("b1h", w['cr_b1'][half * 128:(half + 1) * 128])
    putf("b2h", w['cr_b2'][half * 128:(half + 1) * 128])
    putf("b3h0", w['cr_b3'][half * 256:half * 256 + 128])
    putf("b3h1", w['cr_b3'][half * 256 + 128:half * 256 + 256])
    putf("mtop", np.full(128, float(half), np.float32))
    putf("mbot", np.full(128, float(1 - half), np.float32))
    c0, cols = _LAY_F["identf"]
    constf[:, c0:c0 + 128] = np.eye(128, dtype=np.float32)
    out['constf'] = constf
    return out


def _prep_doc(x, att, mi, ls):
    out = {}
    mif = mi.reshape(EM)
    attm = np.ascontiguousarray(
        att[:, mif, :].transpose(1, 0, 2).reshape(EMH, C))
    amp = np.zeros((128, 9 * C), np.float32)
    for kc in range(9):
        r = min(128, EMH - kc * 128)
        amp[0:r, kc * C:kc * C + C] = attm[kc * 128:kc * 128 + r]
    out['amp'] = np.ascontiguousarray(amp.astype(ml_dtypes.float8_e4m3))
    idx = ls[:, None] + np.arange(SPAN)
    idxf = idx.reshape(LS)
    rows = att[:, idxf, :].reshape(H, L, SPAN, C)
    blocks = np.take_along_axis(rows, idx[None, :, None, :], axis=3)
    attl = blocks.transpose(0, 2, 1, 3).reshape(HS, LS)
    xmT = x[mif].T
    xspr = np.zeros((LS, XSPC), np.float32)
    xspr[:, 0:HID] = x[idxf]
    xspr[:, HID] = 1.0
    actb = np.zeros((128, _CA), np.float32)

    def put(nm, arr):
        c0, cols = _LAY_A[nm]
        actb[0:arr.shape[0], c0:c0 + arr.shape[1]] = arr
    for kc in range(6):
        put(f"xmT{kc}", xmT[kc * 128:(kc + 1) * 128])
    for mc in range(4):
        put(f"xsp{mc}", xspr[mc * 128:(mc + 1) * 128])
    for kc in range(3):
        put(f"attl{kc}", attl[kc * 128:(kc + 1) * 128])
    out['actb'] = _bf(actb)
    xpk = np.zeros((128, 8 * HID), np.float32)
    for kc in range(8):
        xpk[:, kc * HID:(kc + 1) * HID] = x[kc * 128:(kc + 1) * 128]
    out['xp'] = _bf(xpk)
    return out


def build_in_maps(inputs):
    w = {}
    for k, v in inputs.items():
        a = np.asarray(v)
        w[k] = a if a.dtype in (np.int32, np.int64) else \
            np.asarray(a, np.float32)
    shared = _prep_shared(w)
    constf_base = shared.pop('constf_base')
    halves = [_prep_conv_half(w, h, constf_base) for h in range(2)]
    seq = np.asarray(inputs['sequence_output'], np.float32)
    att = np.asarray(inputs['attention'], np.float32)
    mi = np.asarray(inputs['mention_idx']).astype(np.int64)
    ls = np.asarray(inputs['link_start']).astype(np.int64)
    docs = [_prep_doc(seq[n], att[n], mi[n], ls[n]) for n in range(NB)]
    in_maps = []
    for core in range(N_CORES):
        n, half = core // 2, core % 2
        m = dict(shared)
        m.update(halves[half])
        m.update(docs[n])
        in_maps.append({k: (np.ascontiguousarray(v)
                            if v.dtype in (ml_dtypes.bfloat16,
                                           ml_dtypes.float8_e4m3)
                            else np.ascontiguousarray(v, np.float32))
                        for k, v in m.items()})
    return in_maps


def kernel(**inputs):
    nc = _get_program()
    in_maps = build_in_maps(inputs)
    res = run_bass_kernel_spmd(nc, in_maps, list(range(N_CORES)))
    out = np.zeros((NB, EMB, E, E), np.float32)
    for core in range(N_CORES):
        n, half = core // 2, core % 2
        out[n, half * 256:(half + 1) * 256] = \
            res.results[core]["out"].reshape(256, E, E)
    return out

